# revision 12
# baseline (speedup 1.0000x reference)
# CRF Viterbi decode kernel for Trainium2 (Bass/Tile), 8-core data-parallel.
#
# Problem: B=512, T=512, S=64.  reference() = _viterbi_decode:
#   fv_0 = [-1e4 ... except col0=0];  feats_i = logits[:, i+1, :] * masks[:, i+1]
#   step i (0..T-2): scores[b,n,p] = fv[b,p] + trans[n,p]
#                    bptr_i = argmax_p scores (first index wins)
#                    vmax_i = max_p scores;  fv <- vmax_i + feats_i
#   path_score = vmax_{T-2}[:, S-1];  backtrace from bptr_{T-2}[:, S-1].
#
# Device (per core, 64 batches): partitions m = (next-half nh, batch b) = 128;
# free = (next-low nlo 32, prev pl 64) = 2048.  TensorE computes
#   scores[m, nlo, pl] = fv[b, pl] + trans[nh*32+nlo, pl]
# as one K=66 fp32 matmul (64 fv rows selected by an identity table + 2
# next-half indicator rows carrying the transition data), accumulated in the
# reference's rounding order.  DVE reduce-max over pl gives the exact vmax,
# streamed to DRAM.  The fv update (vmax + feat) runs as two PE transposes +
# DVE adds into the weight tile, bit-exact with the reference recursion.
# The backtrace (0.1% of FLOPs) runs on host along the surviving path only,
# using numpy argmax whose first-index tie-break equals jnp.argmax.

import numpy as np

B, T, S = 512, 512, 64
N_CORES = 8
B_LOC = B // N_CORES          # 64
N_STEPS = T - 1               # 511
NH, NLO, PL = 2, 32, 64       # next = nh*32+nlo, prev = pl
NW = NLO * PL                 # 2048 free elements per step
KW = PL + NH                  # matmul contraction: 64 fv rows + 2 trans rows
NEG_INF = -10000.0
UNROLL = 14


def _host_constants(transitions):
    """Constant operand tables shared by all cores."""
    tr = np.asarray(transitions, dtype=np.float32)
    assert tr.shape == (S, S)
    nlo = np.arange(NLO)
    pl = np.arange(PL)

    rhs_big = np.zeros((KW, NW), np.float32)
    # rows 0..63: [pl'==pl] selection of the fv weight rows
    rhs_big[:PL] = np.broadcast_to(
        pl[:, None, None] == pl[None, None, :], (PL, NLO, PL)) \
        .astype(np.float32).reshape(PL, NW)
    # rows 64..65: transition data per next-half (k-order: fv first, trans
    # second => psum accumulates RN(fv) + trans, matching the reference)
    for j in range(NH):
        rhs_big[PL + j] = tr[j * NLO + nlo[:, None], pl[None, :]].reshape(NW)

    lhs0 = np.zeros((KW, 128), np.float32)
    lhs0[:PL] = NEG_INF                       # fv_0 transposed, dup over nh
    lhs0[0, :] = 0.0
    lhs0[PL + 0, 0:64] = 1.0                  # [m // 64 == j] indicators
    lhs0[PL + 1, 64:128] = 1.0
    ident = np.eye(128, dtype=np.float32)
    return dict(rhs_big=rhs_big, lhs0=lhs0, ident=ident)


def build_nc(n_steps=N_STEPS, use_for_i=True, unroll=UNROLL):
    """Build the per-core Bass program (SPMD: same program, per-core data)."""
    from contextlib import ExitStack
    import concourse.bacc as bacc
    import concourse.mybir as mybir
    import concourse.tile as tile
    from concourse.bass import ds

    f32 = mybir.dt.float32
    AX = mybir.AxisListType.X
    MAX = mybir.AluOpType.max
    ADD = mybir.AluOpType.add

    n_loop = 0
    if use_for_i:
        n_loop = max(0, (n_steps - 2) // unroll) * unroll
    n_peel = n_steps - 1 - n_loop              # static steps after the loop
    assert n_peel >= 1 and 1 + n_loop + n_peel == n_steps

    nc = bacc.Bacc("TRN2", target_bir_lowering=False, debug=False)

    d_logitsT = nc.dram_tensor("logits_t", [S, n_steps * B_LOC], f32,
                               kind="ExternalInput")
    d_cons = {
        name: nc.dram_tensor(name, list(shape), f32, kind="ExternalInput")
        for name, shape in [
            ("rhs_big", (KW, NW)), ("lhs0", (KW, 128)), ("ident", (128, 128)),
        ]
    }
    d_vmax = nc.dram_tensor("vmax_out", [128, n_steps * NLO], f32,
                            kind="ExternalOutput")

    with ExitStack() as ctx:
        tc = ctx.enter_context(tile.TileContext(nc))
        consts = ctx.enter_context(tc.tile_pool(name="consts", bufs=1))
        ps_pool = ctx.enter_context(tc.tile_pool(name="ps", bufs=3, space="PSUM"))
        vx_pool = ctx.enter_context(tc.tile_pool(name="vx", bufs=2, space="PSUM"))

        sb = {}
        for name, t in d_cons.items():
            sb[name] = consts.tile(list(t.shape), f32, tag=name, name="sb_" + name)
            nc.sync.dma_start(out=sb[name], in_=t.ap())
        featbuf0 = consts.tile([S, B_LOC], f32, tag="featbuf0")
        featbuf = consts.tile([S, unroll * B_LOC], f32, tag="featbuf")
        featbuf_p = consts.tile([S, max(n_peel, 1) * B_LOC], f32,
                                tag="featbuf_p")
        nc.sync.dma_start(out=featbuf0, in_=d_logitsT.ap()[:, 0:B_LOC])
        # fixed weight tile: rows 0..63 fv (rewritten per step), 64..65 const
        lhs_big = sb["lhs0"]
        # vmax accumulation strips (fixed, DMA'd out per loop body)
        strip0 = consts.tile([128, NLO], f32, tag="strip0")
        strip = consts.tile([128, unroll * NLO], f32, tag="strip")
        strip_p = consts.tile([128, n_peel * NLO], f32, tag="strip_p")

        def emit_step(feat_src, u, m1_out, last):
            """feat_src/u: SBUF feat chunk + static step offset within it."""
            scs = [ps_pool.tile([128, 8, PL], f32, tag="sc", name=f"sc{q}")
                   for q in range(4)]
            for q, scq in enumerate(scs):
                nc.tensor.matmul(scq, lhs_big,
                                 sb["rhs_big"][:, ds(q * 512, 512)],
                                 start=True, stop=True, skip_group_check=True)
                nc.vector.tensor_reduce(m1_out[:, q * 8:(q + 1) * 8], scq,
                                        axis=AX, op=MAX)
            if not last:
                # vmax^T via one PE transpose: Tm[nlo, (nh, b)] at partition 0
                tm = vx_pool.tile([NLO, 128], f32, tag="tm", name="tm")
                nc.tensor.transpose(tm, m1_out[:, 0:NLO], sb["ident"])
                # fv' = vmax + feat into the weight rows
                for h in range(2):
                    tm_sl = tm[:, h * 64:(h + 1) * 64]
                    fe = feat_src[h * 32:(h + 1) * 32,
                                  u * B_LOC:(u + 1) * B_LOC]
                    for d in range(2):
                        nc.vector.tensor_tensor(
                            lhs_big[h * 32:(h + 1) * 32,
                                    d * 64:(d + 1) * 64],
                            tm_sl, fe, op=ADD)

        emit_step(featbuf0, 0, strip0, last=False)
        nc.sync.dma_start(out=d_vmax.ap()[:, 0:NLO], in_=strip0)
        if n_loop:
            with tc.For_i(1, 1 + n_loop, unroll) as i0:
                nc.sync.dma_start(
                    out=featbuf,
                    in_=d_logitsT.ap()[:, ds(i0 * B_LOC, unroll * B_LOC)])
                for u in range(unroll):
                    emit_step(featbuf, u, strip[:, u * NLO:(u + 1) * NLO],
                              last=False)
                nc.sync.dma_start(out=d_vmax.ap()[:, ds(i0 * NLO, unroll * NLO)],
                                  in_=strip)
        if n_peel > 1:
            nc.sync.dma_start(
                out=featbuf_p[:, 0:(n_peel - 1) * B_LOC],
                in_=d_logitsT.ap()[:, (1 + n_loop) * B_LOC:
                                   (n_loop + n_peel) * B_LOC])
        for u in range(n_peel):
            idx = 1 + n_loop + u
            emit_step(featbuf_p, u, strip_p[:, u * NLO:(u + 1) * NLO],
                      last=(idx == n_steps - 1))
        nc.sync.dma_start(
            out=d_vmax.ap()[:, (1 + n_loop) * NLO:n_steps * NLO], in_=strip_p)

    nc.compile()
    return nc


def _host_inputs(logits, masks, transitions, n_steps=N_STEPS):
    """Shard + pre-transpose inputs; returns per-core input maps."""
    logits = np.asarray(logits, dtype=np.float32)
    masks = np.asarray(masks, dtype=np.float32)
    cons = _host_constants(transitions)
    feats = logits[:, 1:1 + n_steps, :] * masks[:, 1:1 + n_steps, None]
    in_maps = []
    for c in range(N_CORES):
        sh = feats[c * B_LOC:(c + 1) * B_LOC]            # [B_LOC, n_steps, S]
        logits_t = np.ascontiguousarray(sh.transpose(2, 1, 0)) \
            .reshape(S, n_steps * B_LOC)                 # [s, (i, b)]
        m = {"logits_t": logits_t}
        m.update(cons)
        in_maps.append(m)
    return in_maps


def _host_decode(results, logits, masks, transitions, n_steps=N_STEPS):
    """path_score + backtrace from the device vmax history.

    The backtrace recomputes each step's backpointer only at the surviving
    tag (64 adds + one argmax per batch per step); np.argmax's first-index
    tie-break matches jnp.argmax, and fv_t = RN(vmax + feat) reproduces the
    device/reference recursion bit-for-bit."""
    logits = np.asarray(logits, dtype=np.float32)
    masks = np.asarray(masks, dtype=np.float32)
    tr = np.asarray(transitions, dtype=np.float32)

    vmaxs = np.empty((n_steps, B, S), np.float32)        # [i, b, next]
    for c, res in enumerate(results):
        vh = res["vmax_out"].reshape(128, n_steps, NLO)
        for nh in range(NH):
            vmaxs[:, c * B_LOC:(c + 1) * B_LOC, nh * NLO:(nh + 1) * NLO] = \
                vh[nh * 64:(nh + 1) * 64].transpose(1, 0, 2)
    path_score = vmaxs[n_steps - 1, :, S - 1].copy()

    feats = logits[:, 1:1 + n_steps, :] * masks[:, 1:1 + n_steps, None]

    def fv_at(i):
        if i == 0:
            fv = np.full((B, S), NEG_INF, np.float32)
            fv[:, 0] = 0.0
            return fv
        return vmaxs[i - 1] + feats[:, i - 1, :]

    seq = np.empty((B, n_steps), np.int32)
    fv = fv_at(n_steps - 1)
    tag = np.argmax(fv + tr[S - 1][None, :], axis=1)     # t0 seed
    seq[:, n_steps - 1] = tag
    for i in range(n_steps - 1, 0, -1):
        if i != n_steps - 1:
            fv = fv_at(i)
        tag = np.argmax(fv + tr[tag, :], axis=1)
        seq[:, i - 1] = tag
    return path_score, seq


_NC_CACHE = {}


def kernel(logits, masks, transitions):
    from concourse.bass_utils import run_bass_kernel_spmd

    key = N_STEPS
    if key not in _NC_CACHE:
        _NC_CACHE[key] = build_nc(N_STEPS)
    nc = _NC_CACHE[key]
    in_maps = _host_inputs(logits, masks, transitions)
    res = run_bass_kernel_spmd(nc, in_maps, core_ids=list(range(N_CORES)))
    return _host_decode(res.results, logits, masks, transitions)
